# revision 15
# baseline (speedup 1.0000x reference)
"""Trainium2 Bass kernel for nn_Actor (pointer-network actor: encoder LSTM over
S=1025 items + 1025-step attention decode with masked argmax).

Strategy: the recurrences are strictly sequential with tiny per-step matvecs, so
per-step cross-core collectives (~5-10us floor) can never pay off.  All 8 cores
run the same single-core fused kernel (SPMD replicate); core 0's output is used.
Inside one core the work is laid out to keep the serial chain short:
  - state h,c kept partition-major [128,2]; gates computed as [1,1024] row via
    PE streaming matmuls, then 8 PE transposes -> [128,8] for ACT/DVE postwork
  - attention scores computed directly partition-major [128,9] via 18
    weight-stationary matmuls (lhsT = tanh(ujT) chunks, rhs = V)
  - softmax without max-subtraction (scores are O(+-10)); di normalized once
  - masked argmax via iota-compare-min; mask updates via copy_predicated
  - log-softmax ln(sum) via exponent-bits initial guess + 2 Newton iterations
    (keeps every ACT call inside the exp_and_others table set: exp+tanh only;
    sigmoid is computed as 0.5*(1+tanh(x/2)) with the g-gate weight rows
    pre-scaled by 2 so one ACT tanh(x*0.5) op serves all four gates)
"""

import sys, os

sys.path.insert(0, "/opt/trn_rl_repo")

import numpy as np
import concourse.bass as bass
import concourse.bacc as bacc
import concourse.tile as tile
from concourse import mybir

F32 = mybir.dt.float32
I32 = mybir.dt.int32
U8 = mybir.dt.uint8
AF = mybir.ActivationFunctionType
OP = mybir.AluOpType
AX = mybir.AxisListType

N, H, A, E = 1024, 256, 256, 16
S = N + 1            # 1025
C9 = 9               # ceil(S/128)
SP = C9 * 128        # 1152 padded S
ENCW = 1 + SP        # enc_outT width: col 0 = zero initial h, col t+1 = h_t
G4 = 4 * H           # 1024 gate width
DEC_UPTO = int(os.environ.get("DEC_UPTO", "99"))
LN2 = 0.6931471805599453
K_BITS = LN2 / (1 << 23)
C_BITS = -127.0 * LN2


def build(n_enc=S, n_dec=S, probs_name="probs", preds_name="preds"):
    nc = bacc.Bacc("TRN2", target_bir_lowering=False, debug=False)

    # ---------------- DRAM I/O ----------------
    d_items = nc.dram_tensor("items", [1, N, 2], F32, kind="ExternalInput").ap()
    d_wemb = nc.dram_tensor("W_emb", [E, 2], F32, kind="ExternalInput").ap()
    d_ewih = nc.dram_tensor("enc_Wih", [G4, E], F32, kind="ExternalInput").ap()
    d_ewhh = nc.dram_tensor("enc_Whh", [G4, H], F32, kind="ExternalInput").ap()
    d_ebih = nc.dram_tensor("enc_bih", [G4], F32, kind="ExternalInput").ap()
    d_ebhh = nc.dram_tensor("enc_bhh", [G4], F32, kind="ExternalInput").ap()
    d_dwih = nc.dram_tensor("dec_Wih", [G4, H + 2], F32, kind="ExternalInput").ap()
    d_dwhh = nc.dram_tensor("dec_Whh", [G4, H], F32, kind="ExternalInput").ap()
    d_dbih = nc.dram_tensor("dec_bih", [G4], F32, kind="ExternalInput").ap()
    d_dbhh = nc.dram_tensor("dec_bhh", [G4], F32, kind="ExternalInput").ap()
    d_w1 = nc.dram_tensor("W1", [A, H], F32, kind="ExternalInput").ap()
    d_w2 = nc.dram_tensor("W2", [A, H], F32, kind="ExternalInput").ap()
    d_v = nc.dram_tensor("V", [A], F32, kind="ExternalInput").ap()
    d_probs = nc.dram_tensor(probs_name, [S, S], F32, kind="ExternalOutput").ap()
    d_preds = nc.dram_tensor(preds_name, [S], I32, kind="ExternalOutput").ap()

    with tile.TileContext(nc) as tc:
        _emit(nc, tc, locals(), n_enc, n_dec)
    nc.compile()
    return nc


def _emit(nc, tc, d, n_enc, n_dec):
    d_items = d["d_items"]; d_wemb = d["d_wemb"]
    d_ewih = d["d_ewih"]; d_ewhh = d["d_ewhh"]; d_ebih = d["d_ebih"]; d_ebhh = d["d_ebhh"]
    d_dwih = d["d_dwih"]; d_dwhh = d["d_dwhh"]; d_dbih = d["d_dbih"]; d_dbhh = d["d_dbhh"]
    d_w1 = d["d_w1"]; d_w2 = d["d_w2"]; d_v = d["d_v"]
    d_probs = d["d_probs"]; d_preds = d["d_preds"]

    ctx_pools = []

    def pool(name, bufs, space="SBUF"):
        p = tc.tile_pool(name=name, bufs=bufs, space=space)
        ctx_pools.append(p)
        return p.__enter__()

    P1 = pool("persist", 1)          # persistent uniquely-tagged tiles
    PS = pool("step", 2)             # per-step cycled scratch
    PP = pool("psum", 1, "PSUM")     # psum: explicit tags, 8 banks total
    PT = pool("tiny_ps", 2, "PSUM")  # small psum tiles share one tag

    def ptile(shape, tag, dtype=F32):
        return PP.tile(shape, dtype, tag=tag, name=tag)

    def tiny(shape, dtype=F32):
        return PT.tile(shape, dtype, tag="tiny", name="tiny")

    V_ = nc.vector
    SC = nc.scalar
    TE = nc.tensor
    GP = nc.gpsimd
    SY = nc.sync

    # =====================================================================
    # constants + identities
    # =====================================================================
    id128 = P1.tile([128, 128], F32, tag="id128", name="id128")
    it1 = P1.tile([128, 128], I32, tag="it1", name="it1")
    it2 = P1.tile([128, 128], I32, tag="it2", name="it2")
    GP.iota(it1, pattern=[[0, 128]], base=0, channel_multiplier=1)   # = p
    GP.iota(it2, pattern=[[1, 128]], base=0, channel_multiplier=0)   # = col
    f1 = P1.tile([128, 128], F32, tag="f1", name="f1")
    f2 = P1.tile([128, 128], F32, tag="f2", name="f2")
    V_.tensor_copy(f1, it1)
    V_.tensor_copy(f2, it2)
    V_.tensor_tensor(id128, f1, f2, OP.is_equal)

    id1 = P1.tile([1, 1], F32, tag="id1", name="id1")
    V_.memset(id1, 1.0)

    iotai = P1.tile([128, C9], I32, tag="iotai", name="iotai")
    GP.iota(iotai, pattern=[[128, C9]], base=0, channel_multiplier=1)  # j = c*128+p
    iotaf = P1.tile([128, C9], F32, tag="iotaf", name="iotaf")
    V_.tensor_copy(iotaf, iotai)

    tinyv = P1.tile([128, C9], F32, tag="tinyv", name="tinyv")
    V_.memset(tinyv, -3e-37)
    V_.memset(tinyv[:, 8:9], -1e38)              # pads j>=1025...
    V_.memset(tinyv[0:1, 8:9], -3e-37)           # ...but j=1024 is valid
    hugev = P1.tile([128, C9], F32, tag="hugev", name="hugev")
    V_.memset(hugev, -3e37)
    padneg = P1.tile([128, C9], F32, tag="padneg", name="padneg")
    V_.memset(padneg, 0.0)
    V_.memset(padneg[:, 8:9], -1e38)
    V_.memset(padneg[0:1, 8:9], 0.0)
    bigc = P1.tile([128, C9], F32, tag="bigc", name="bigc")
    V_.memset(bigc, 1e9)
    zeroc = P1.tile([128, C9], U8, tag="zeroc", name="zeroc")
    V_.memset(zeroc, 0)
    onec = P1.tile([128, C9], U8, tag="onec", name="onec")
    V_.memset(onec, 1)
    e0 = P1.tile([128, C9], U8, tag="e0", name="e0")
    V_.memset(e0, 0)
    V_.memset(e0[0:1, 0:1], 1)
    onesP = P1.tile([128, 1], F32, tag="onesP", name="onesP")
    V_.memset(onesP, 1.0)
    ones1 = P1.tile([1, 128], F32, tag="ones1", name="ones1")
    V_.memset(ones1, 1.0)

    maskb = P1.tile([128, C9], U8, tag="maskb", name="maskb")   # mask state (1=avail)
    V_.memset(maskb, 1)
    V_.memset(maskb[0:1, 0:1], 0)                  # depot masked at t=0
    V_.memset(maskb[:, 8:9], 0)                    # pads...
    V_.memset(maskb[0:1, 8:9], 1)                  # ...but j=1024 valid

    # =====================================================================
    # weight prep (DMA natural layout, transpose on PE)
    # =====================================================================
    STG = pool("staging", 1)

    def load_pm(dram_ap, rows, cols, name):
        """[rows, cols] dram -> sbuf [128, rows//128, cols] partition-major."""
        t = STG.tile([128, rows // 128, cols], F32, tag=name)
        SY.dma_start(out=t, in_=dram_ap.rearrange("(rc p) c -> p rc c", p=128))
        return t

    def transp128(dst_ap, src_ap):
        """transpose [p,f]<=128 sbuf block -> dst sbuf [f,p] via PE + copy."""
        pf = src_ap.shape[0]
        ff = src_ap.free_size()
        tp = tiny([ff, pf])
        TE.matmul(tp, src_ap, id128[0:pf, 0:pf], is_transpose=True)
        V_.tensor_copy(dst_ap, tp)

    # encoder weights: WencT_h [128,2,G4] (Whh.T), WencT_e [17,G4] (Wih.T + bias)
    whh_s = load_pm(d_ewhh, G4, H, "whh_s")
    WencT_h = P1.tile([128, 2, G4], F32, tag="WencT_h", name="WencT_h")
    for rc in range(8):
        for kt in range(2):
            transp128(WencT_h[:, kt, rc * 128:(rc + 1) * 128],
                      whh_s[:, rc, kt * 128:(kt + 1) * 128])
    wih_s = load_pm(d_ewih, G4, E, "wih_s")
    b1 = STG.tile([128, 8], F32, tag="b1", name="b1")
    b2 = STG.tile([128, 8], F32, tag="b2", name="b2")
    SY.dma_start(out=b1, in_=d_ebih.rearrange("(rc p) -> p rc", p=128))
    SY.dma_start(out=b2, in_=d_ebhh.rearrange("(rc p) -> p rc", p=128))
    xe_s = STG.tile([128, 8, 17], F32, tag="xe_s", name="xe_s")
    V_.tensor_copy(xe_s[:, :, 0:E], wih_s)
    V_.tensor_tensor(xe_s[:, :, 16], b1, b2, OP.add)
    WencT_e = P1.tile([17, G4], F32, tag="WencT_e", name="WencT_e")
    for rc in range(8):
        transp128(WencT_e[:, rc * 128:(rc + 1) * 128], xe_s[:, rc, :])
    # g-gate (cols 512:768) rows x2 so a single tanh(x*0.5) serves all gates
    V_.tensor_scalar_mul(WencT_h[:, :, 512:768], WencT_h[:, :, 512:768], 2.0)
    V_.tensor_scalar_mul(WencT_e[:, 512:768], WencT_e[:, 512:768], 2.0)

    # decoder weights: Wd_di0/1, Wd_h0/1 [128,G4]; Wd_tail [3,G4]
    dwih_s = load_pm(d_dwih, G4, H + 2, "dwih_s")
    Wd_di = P1.tile([128, 2, G4], F32, tag="Wd_di", name="Wd_di")
    for rc in range(8):
        for kt in range(2):
            transp128(Wd_di[:, kt, rc * 128:(rc + 1) * 128],
                      dwih_s[:, rc, kt * 128:(kt + 1) * 128])
    b3 = STG.tile([128, 8], F32, tag="b1", name="b3")
    b4 = STG.tile([128, 8], F32, tag="b2", name="b4")
    SY.dma_start(out=b3, in_=d_dbih.rearrange("(rc p) -> p rc", p=128))
    SY.dma_start(out=b4, in_=d_dbhh.rearrange("(rc p) -> p rc", p=128))
    xd_s = STG.tile([128, 8, 3], F32, tag="xd_s", name="xd_s")
    V_.tensor_copy(xd_s[:, :, 0:2], dwih_s[:, :, 256:258])
    V_.tensor_tensor(xd_s[:, :, 2], b3, b4, OP.add)
    Wd_tail = P1.tile([3, G4], F32, tag="Wd_tail", name="Wd_tail")
    for rc in range(8):
        transp128(Wd_tail[:, rc * 128:(rc + 1) * 128], xd_s[:, rc, :])
    dwhh_s = load_pm(d_dwhh, G4, H, "dwhh_s")
    Wd_h = P1.tile([128, 2, G4], F32, tag="Wd_h", name="Wd_h")
    for rc in range(8):
        for kt in range(2):
            transp128(Wd_h[:, kt, rc * 128:(rc + 1) * 128],
                      dwhh_s[:, rc, kt * 128:(kt + 1) * 128])
    V_.tensor_scalar_mul(Wd_di[:, :, 512:768], Wd_di[:, :, 512:768], 2.0)
    V_.tensor_scalar_mul(Wd_h[:, :, 512:768], Wd_h[:, :, 512:768], 2.0)
    V_.tensor_scalar_mul(Wd_tail[:, 512:768], Wd_tail[:, 512:768], 2.0)

    # W1T / W2T [128, 2, 256]: [p_h, kt_h, a] = W[a, kt*128+p]
    w1_s = load_pm(d_w1, A, H, "w1_s")
    W1T = P1.tile([128, 2, A], F32, tag="W1T", name="W1T")
    for rc in range(2):
        for kt in range(2):
            transp128(W1T[:, kt, rc * 128:(rc + 1) * 128],
                      w1_s[:, rc, kt * 128:(kt + 1) * 128])
    w2_s = load_pm(d_w2, A, H, "w2_s")
    W2T = P1.tile([128, 2, A], F32, tag="W2T", name="W2T")
    for rc in range(2):
        for kt in range(2):
            transp128(W2T[:, kt, rc * 128:(rc + 1) * 128],
                      w2_s[:, rc, kt * 128:(kt + 1) * 128])

    # V [256] -> V2 [128,2]
    v_row = STG.tile([1, A], F32, tag="v_row", name="v_row")
    SY.dma_start(out=v_row, in_=d_v.rearrange("(a n) -> a n", a=1))
    V2 = P1.tile([128, 2], F32, tag="V2", name="V2")
    for kt in range(2):
        transp128(V2[:, kt:kt + 1], v_row[0:1, kt * 128:(kt + 1) * 128])

    # items natural layout -> itT [2, SP] via per-chunk transposes (depot col 0 = 0)
    items_s = STG.tile([128, 8, 2], F32, tag="items_s")
    SY.dma_start(out=items_s, in_=d_items[0].rearrange("(rc p) c -> p rc c", p=128))
    itT = P1.tile([2, SP], F32, tag="itT", name="itT")
    V_.memset(itT, 0.0)
    for rc in range(8):
        transp128(itT[0:2, 1 + rc * 128:1 + (rc + 1) * 128], items_s[:, rc, :])
    # items j-major [128, C9, 2]: it2jm[p,c,:] = items_full[c*128+p]
    it2jm = P1.tile([128, C9, 2], F32, tag="it2jm", name="it2jm")
    V_.memset(it2jm, 0.0)
    for c in range(C9):
        transp128(it2jm[:, c, :], itT[0:2, c * 128:(c + 1) * 128])

    # W_emb [16,2] -> WeT [2,16]; emb_aug [17, SP] rows0-15 = W_emb @ items_fullT
    wemb_s = STG.tile([E, 2], F32, tag="wemb_s", name="wemb_s")
    SY.dma_start(out=wemb_s, in_=d_wemb)
    WeT = P1.tile([2, E], F32, tag="WeT", name="WeT")
    transp128(WeT, wemb_s)
    emb_aug = P1.tile([17, SP], F32, tag="emb_aug", name="emb_aug")
    V_.memset(emb_aug, 1.0)
    for jn in range(3):
        sl = slice(jn * 384, (jn + 1) * 384)
        ep = ptile([E, 384], "sps")
        TE.matmul(ep, WeT, itT[:, sl], start=True, stop=True)
        V_.tensor_copy(emb_aug[0:E, sl], ep)

    # =====================================================================
    # state
    # =====================================================================
    enc_outT = P1.tile([128, 2, ENCW], F32, tag="enc_outT", name="enc_outT")
    V_.memset(enc_outT, 0.0)
    c_st = P1.tile([128, 2], F32, tag="c_st", name="c_st")
    V_.memset(c_st, 0.0)

    def lstm_post(g_ps, h_out_ap, c_tile):
        """gates psum [1,1024] -> transpose -> tanh -> c/h update.
        h_out_ap: [128,2] destination for new h."""
        g_sb = PS.tile([1, G4], F32, tag="g_sb", name="g_sb")
        V_.tensor_copy(g_sb[0:1, 0:512], g_ps[0:1, 0:512])
        SC.copy(g_sb[0:1, 512:G4], g_ps[0:1, 512:G4])
        gpm = ptile([128, 8], "qps")
        for gc in range(8):
            TE.matmul(gpm[:, gc:gc + 1], g_sb[0:1, gc * 128:(gc + 1) * 128],
                      id1, is_transpose=True)
        tg8 = PS.tile([128, 8], F32, tag="tg8", name="tg8")
        SC.activation(tg8, gpm, AF.Tanh, scale=0.5)
        ti, tf, tg, to = tg8[:, 0:2], tg8[:, 2:4], tg8[:, 4:6], tg8[:, 6:8]
        # c2 = 0.5*(c + tf*c + tg + ti*tg); h2 = 0.5*(tanh(c2) + to*tanh(c2))
        a_ = PS.tile([128, 2], F32, tag="ls_a", name="ls_a")
        V_.tensor_tensor(a_, tf, c_tile, OP.mult)
        dd = PS.tile([128, 2], F32, tag="ls_d", name="ls_d")
        V_.tensor_tensor(dd, c_tile, tg, OP.add)
        bb = PS.tile([128, 2], F32, tag="ls_b", name="ls_b")
        V_.tensor_tensor(bb, ti, tg, OP.mult)
        s1 = PS.tile([128, 2], F32, tag="ls_s1", name="ls_s1")
        V_.tensor_tensor(s1, a_, bb, OP.add)
        s2 = PS.tile([128, 2], F32, tag="ls_s2", name="ls_s2")
        V_.tensor_tensor(s2, s1, dd, OP.add)
        V_.tensor_scalar_mul(c_tile, s2, 0.5)
        tc_ = PS.tile([128, 2], F32, tag="ls_tc", name="ls_tc")
        SC.activation(tc_, c_tile, AF.Tanh)
        mm_ = PS.tile([128, 2], F32, tag="ls_m", name="ls_m")
        V_.tensor_tensor(mm_, to, tc_, OP.mult)
        s3 = PS.tile([128, 2], F32, tag="ls_s3", name="ls_s3")
        V_.tensor_tensor(s3, tc_, mm_, OP.add)
        V_.tensor_scalar_mul(h_out_ap, s3, 0.5)

    # =====================================================================
    # encoder: 1025 steps
    # =====================================================================
    for t in range(n_enc):
        g_ps = ptile([1, G4], "gps")
        for nh in range(2):
            osl = slice(nh * 512, (nh + 1) * 512)
            TE.matmul(g_ps[0:1, osl], enc_outT[:, 0, t:t + 1], WencT_h[:, 0, osl],
                      start=True, stop=False)
            TE.matmul(g_ps[0:1, osl], enc_outT[:, 1, t:t + 1], WencT_h[:, 1, osl],
                      start=False, stop=False)
            TE.matmul(g_ps[0:1, osl], emb_aug[:, t:t + 1], WencT_e[:, osl],
                      start=False, stop=True)
        lstm_post(g_ps, enc_outT[:, :, t + 1], c_st)

    # =====================================================================
    # post-encoder: enc_out_jm [128,C9,H], enc_projT [128,2,SP]
    # =====================================================================
    enc_jm = P1.tile([128, C9, H], F32, tag="enc_jm", name="enc_jm")
    V_.memset(enc_jm, 0.0)
    for jt in range(C9):
        for ht in range(2):
            transp128(enc_jm[:, jt, ht * 128:(ht + 1) * 128],
                      enc_outT[:, ht, 1 + jt * 128:1 + (jt + 1) * 128])
    enc_projT = P1.tile([128, 2, SP], F32, tag="enc_projT", name="enc_projT")
    for ac in range(2):
        for jn in range(3):
            sl = slice(jn * 384, (jn + 1) * 384)
            pp_ = ptile([128, 384], "sps")
            TE.matmul(pp_, W1T[:, 0, ac * 128:(ac + 1) * 128],
                      enc_outT[:, 0, 1 + jn * 384:1 + (jn + 1) * 384],
                      start=True, stop=False)
            TE.matmul(pp_, W1T[:, 1, ac * 128:(ac + 1) * 128],
                      enc_outT[:, 1, 1 + jn * 384:1 + (jn + 1) * 384],
                      start=False, stop=True)
            V_.tensor_copy(enc_projT[:, ac, sl], pp_)

    # decode state
    h_st = P1.tile([128, 2], F32, tag="h_st", name="h_st")
    V_.tensor_copy(h_st, enc_outT[:, :, n_enc])
    xh3 = P1.tile([3, 1], F32, tag="xh3", name="xh3")
    V_.memset(xh3, 1.0)
    V_.memset(xh3[0:2, 0:1], 0.0)
    preds_sb = P1.tile([1, S], I32, tag="preds_sb", name="preds_sb")
    V_.memset(preds_sb, 0)
    tail_sb = P1.tile([1, S], F32, tag="tail_sb", name="tail_sb")
    V_.memset(tail_sb, 0.0)

    # =====================================================================
    # decoder: 1025 steps
    # =====================================================================
    for t in range(n_dec):
        # --- q = W2 @ h, partition-major [128,2]
        q_ps = ptile([128, 2], "qps")
        for ac in range(2):
            TE.matmul(q_ps[:, ac:ac + 1], W2T[:, 0, ac * 128:(ac + 1) * 128],
                      h_st[:, 0:1], start=True, stop=False)
            TE.matmul(q_ps[:, ac:ac + 1], W2T[:, 1, ac * 128:(ac + 1) * 128],
                      h_st[:, 1:2], start=False, stop=True)
        q_sb = PS.tile([128, 2], F32, tag="q_sb", name="q_sb")
        V_.tensor_copy(q_sb, q_ps)
        if DEC_UPTO < 2: continue

        # --- ujT = tanh(enc_projT + q)
        ujT = PS.tile([128, 2, SP], F32, tag="ujT", name="ujT")
        for half in range(2):
            SC.activation(ujT[:, half, :], enc_projT[:, half, :], AF.Tanh,
                          bias=q_sb[:, half:half + 1])

        # --- scores [128,C9] partition-major (+ -1e38 pads)
        if DEC_UPTO < 3: continue
        sc_ps = ptile([128, C9], "sps")
        for jc in range(C9):
            TE.matmul(sc_ps[:, jc:jc + 1], ujT[:, 0, jc * 128:(jc + 1) * 128],
                      V2[:, 0:1], start=True, stop=False)
            TE.matmul(sc_ps[:, jc:jc + 1], ujT[:, 1, jc * 128:(jc + 1) * 128],
                      V2[:, 1:2], start=False, stop=True)
        scores = PS.tile([128, C9], F32, tag="scores", name="scores")
        V_.tensor_tensor(scores, sc_ps, padneg, OP.add)

        # --- softmax (no max-sub) -> di
        if DEC_UPTO < 4: continue
        expv = PS.tile([128, C9], F32, tag="expv", name="expv")
        sumP = PS.tile([128, 1], F32, tag="sumP", name="sumP")
        SC.activation(expv, scores, AF.Exp, accum_out=sumP)
        ssum = tiny([1, 1])
        TE.matmul(ssum, onesP, sumP, start=True, stop=True)
        invS = PS.tile([1, 1], F32, tag="invS", name="invS")
        V_.reciprocal(invS, ssum)
        if DEC_UPTO < 5: continue
        di_ps = ptile([1, H], "dips")
        for jt in range(C9):
            TE.matmul(di_ps, expv[:, jt:jt + 1], enc_jm[:, jt, :],
                      start=(jt == 0), stop=(jt == C9 - 1))
        di_n = PS.tile([1, H], F32, tag="di_n", name="di_n")
        V_.tensor_scalar(di_n, di_ps, invS, None, OP.mult)
        xh_di = PS.tile([128, 2], F32, tag="xh_di", name="xh_di")
        for cc in range(2):
            dT = tiny([128, 1])
            TE.matmul(dT, di_n[0:1, cc * 128:(cc + 1) * 128], id1, is_transpose=True)
            V_.tensor_copy(xh_di[:, cc:cc + 1], dT)

        if DEC_UPTO < 6: continue
        # --- masked vectors (read maskb BEFORE update)
        y = PS.tile([128, C9], F32, tag="y", name="y")
        V_.tensor_copy(y, tinyv)
        V_.copy_predicated(y, maskb, scores)
        z = PS.tile([128, C9], F32, tag="z", name="z")
        V_.tensor_copy(z, hugev)
        V_.copy_predicated(z, maskb, scores)

        if DEC_UPTO < 7: continue
        # --- argmax(z) -> pred (first-index tiebreak via iota min)
        zmax = PS.tile([128, 1], F32, tag="zmax", name="zmax")
        V_.tensor_reduce(zmax, z, AX.X, OP.max)
        zmT = tiny([1, 128])
        TE.matmul(zmT, zmax, id128, is_transpose=True)
        gmax = PS.tile([1, 1], F32, tag="gmax", name="gmax")
        V_.tensor_reduce(gmax, zmT, AX.X, OP.max)
        gb_ps = tiny([128, 1])
        TE.matmul(gb_ps, ones1, gmax, start=True, stop=True)
        gmb = PS.tile([128, 1], F32, tag="gmb", name="gmb")
        V_.tensor_copy(gmb, gb_ps)
        eqm = PS.tile([128, C9], U8, tag="eqm", name="eqm")
        V_.tensor_scalar(eqm, z, gmb, None, OP.is_equal)
        isel = PS.tile([128, C9], F32, tag="isel", name="isel")
        V_.tensor_copy(isel, bigc)
        V_.copy_predicated(isel, eqm, iotaf)
        imin = PS.tile([128, 1], F32, tag="imin", name="imin")
        V_.tensor_reduce(imin, isel, AX.X, OP.min)
        imT = tiny([1, 128])
        TE.matmul(imT, imin, id128, is_transpose=True)
        predf = PS.tile([1, 1], F32, tag="predf", name="predf")
        V_.tensor_reduce(predf, imT, AX.X, OP.min)
        V_.tensor_copy(preds_sb[0:1, t:t + 1], predf)

        if DEC_UPTO < 8: continue
        # --- eqpred, dec2 gather, mask update
        pb_ps = tiny([128, 1])
        TE.matmul(pb_ps, ones1, predf, start=True, stop=True)
        pmb = PS.tile([128, 1], F32, tag="pmb", name="pmb")
        V_.tensor_copy(pmb, pb_ps)
        eqp = PS.tile([128, C9], U8, tag="eqp", name="eqp")
        V_.tensor_scalar(eqp, iotaf, pmb, None, OP.is_equal)
        eqpf = PS.tile([128, C9], F32, tag="eqpf", name="eqpf")
        V_.tensor_scalar(eqpf, iotaf, pmb, None, OP.is_equal)
        if DEC_UPTO < 9: continue
        d2s = PS.tile([128, 2], F32, tag="d2s", name="d2s")
        scr = PS.tile([128, C9], F32, tag="scr", name="scr")
        V_.tensor_tensor(scr, eqpf, it2jm[:, :, 0], OP.mult)
        V_.tensor_reduce(d2s[:, 0:1], scr, AX.X, OP.add)
        scr2 = PS.tile([128, C9], F32, tag="scr2", name="scr2")
        V_.tensor_tensor(scr2, eqpf, it2jm[:, :, 1], OP.mult)
        V_.tensor_reduce(d2s[:, 1:2], scr2, AX.X, OP.add)
        d2_ps = tiny([1, 2])
        TE.matmul(d2_ps, onesP, d2s, start=True, stop=True)
        d2row = PS.tile([1, 2], F32, tag="d2row", name="d2row")
        V_.tensor_copy(d2row, d2_ps)
        d2T = tiny([2, 1])
        TE.matmul(d2T, d2row, id1, is_transpose=True)

        if DEC_UPTO < 10: continue
        # mask update: mask[pred]=0 then mask[0]=1
        V_.copy_predicated(maskb, eqp, zeroc)
        V_.copy_predicated(maskb, e0, onec)

        # --- gates (read xh3 = PREVIOUS dec2 before overwriting it)
        if DEC_UPTO < 11: continue
        g_ps = ptile([1, G4], "gps")
        for nh in range(2):
            osl = slice(nh * 512, (nh + 1) * 512)
            TE.matmul(g_ps[0:1, osl], xh_di[:, 0:1], Wd_di[:, 0, osl],
                      start=True, stop=False)
            TE.matmul(g_ps[0:1, osl], xh_di[:, 1:2], Wd_di[:, 1, osl],
                      start=False, stop=False)
            TE.matmul(g_ps[0:1, osl], h_st[:, 0:1], Wd_h[:, 0, osl],
                      start=False, stop=False)
            TE.matmul(g_ps[0:1, osl], h_st[:, 1:2], Wd_h[:, 1, osl],
                      start=False, stop=False)
            TE.matmul(g_ps[0:1, osl], xh3, Wd_tail[:, osl],
                      start=False, stop=True)
        # now overwrite xh3 with this step's dec2 (for next step)
        V_.tensor_copy(xh3[0:2, 0:1], d2T)

        if DEC_UPTO < 12: continue
        lstm_post(g_ps, h_st, c_st)

        if DEC_UPTO < 13: continue
        # --- lp = y - ln(sum(exp(y)))  [off critical path]
        ey = PS.tile([128, C9], F32, tag="ey", name="ey")
        sumY = PS.tile([128, 1], F32, tag="sumY", name="sumY")
        SC.activation(ey, y, AF.Exp, accum_out=sumY)
        sy_ps = tiny([1, 1])
        TE.matmul(sy_ps, onesP, sumY, start=True, stop=True)
        sy = PS.tile([1, 1], F32, tag="sy", name="sy")
        V_.tensor_copy(sy, sy_ps)
        bf = PS.tile([1, 1], F32, tag="bf", name="bf")
        V_.tensor_copy(bf, sy.bitcast(I32))
        zz = PS.tile([1, 3], F32, tag="zz", name="zz")
        V_.tensor_scalar(zz[0:1, 0:1], bf, K_BITS, C_BITS, OP.mult, OP.add)
        for it_ in range(2):
            ezt = PS.tile([1, 1], F32, tag="ezt", name="ezt")
            SC.activation(ezt, zz[0:1, it_:it_ + 1], AF.Exp, scale=-1.0)
            pr = PS.tile([1, 1], F32, tag="pr", name="pr")
            V_.tensor_tensor(pr, ezt, sy, OP.mult)
            V_.scalar_tensor_tensor(zz[0:1, it_ + 1:it_ + 2], pr, -1.0,
                                    zz[0:1, it_:it_ + 1], OP.add, OP.add)
        lnb_ps = tiny([128, 1])
        TE.matmul(lnb_ps, ones1, zz[0:1, 2:3], start=True, stop=True)
        lnb = PS.tile([128, 1], F32, tag="lnb", name="lnb")
        V_.tensor_copy(lnb, lnb_ps)
        lp = PS.tile([128, C9], F32, tag="lp", name="lp")
        V_.tensor_scalar(lp, y, lnb, None, OP.subtract)
        V_.tensor_copy(tail_sb[0:1, t:t + 1], lp[0:1, 8:9])
        lpT = ptile([C9, 128], "sps")
        TE.matmul(lpT, lp, id128, is_transpose=True)
        lpT_sb = PS.tile([C9, 128], F32, tag="lpT_sb", name="lpT_sb")
        V_.tensor_copy(lpT_sb, lpT)
        SY.dma_start(
            out=d_probs[t:t + 1, 0:1024].rearrange("one (c p) -> (one c) p", p=128),
            in_=lpT_sb[0:8, :])

    # final outputs
    SY.dma_start(out=d_preds.rearrange("(a n) -> a n", a=1), in_=preds_sb)
    SY.dma_start(out=d_probs[0:S, 1024:1025].rearrange("s one -> one s"),
                 in_=tail_sb)

    for p in reversed(ctx_pools):
        p.__exit__(None, None, None)


_NC_CACHE = {}


def kernel(**inputs):
    from concourse.bass_utils import run_bass_kernel_spmd

    key = ("full", S, S)
    if key not in _NC_CACHE:
        _NC_CACHE[key] = build()
    nc = _NC_CACHE[key]

    feed = {}
    for name in ["items", "W_emb", "enc_Wih", "enc_Whh", "enc_bih", "enc_bhh",
                 "dec_Wih", "dec_Whh", "dec_bih", "dec_bhh", "W1", "W2", "V"]:
        feed[name] = np.ascontiguousarray(np.asarray(inputs[name], dtype=np.float32))
    in_maps = [dict(feed) for _ in range(8)]
    res = run_bass_kernel_spmd(nc, in_maps, core_ids=list(range(8)))
    probs = np.asarray(res.results[0]["probs"], dtype=np.float32)
    preds = np.asarray(res.results[0]["preds"], dtype=np.int32)
    return probs, preds


def timed_run(np_inputs, trace_dir=None):
    """Run once more with NTFF profiling; returns exec_time_ns (core 0)."""
    from concourse.bass_utils import run_bass_kernel_spmd

    key = ("full", S, S)
    if key not in _NC_CACHE:
        _NC_CACHE[key] = build()
    nc = _NC_CACHE[key]
    feed = {}
    for name in ["items", "W_emb", "enc_Wih", "enc_Whh", "enc_bih", "enc_bhh",
                 "dec_Wih", "dec_Whh", "dec_bih", "dec_bhh", "W1", "W2", "V"]:
        feed[name] = np.ascontiguousarray(np.asarray(np_inputs[name], dtype=np.float32))
    in_maps = [dict(feed) for _ in range(8)]
    res = run_bass_kernel_spmd(nc, in_maps, core_ids=list(range(8)), trace=True,
                               tmpdir=trace_dir)
    return res.exec_time_ns


if __name__ == "__main__":
    nc = build(n_enc=4, n_dec=4)
    print("built OK")
